# revision 1
# baseline (speedup 1.0000x reference)
"""Trainium2 Bass kernel for DSS-GIN conv (gnn_message_passing).

Strategy (8 NeuronCores, B=128 subgraphs sharded 16/core):
  - h = MLP_t(X) computed per-core in transposed space via PE matmuls (fp32r).
  - Pooled node branch: per-core partial max over local subgraphs, AllReduce(max)
    across cores (split into 4 column groups, pipelined behind stage A), then
    nodex = MLP_n(xmax) replicated on every core.
  - Message passing ret1 + broadcast nodex2 folded into ONE dense matmul:
      out[b] = S^T @ (h[b] + nodex)   where S[j,k] = #edges j->k  (built on host
    from edge_index, streamed from DRAM as 128x128 tiles).
  - All matmuls in float32r (full PE rate at free>=256, ~12-bit mantissa).
"""
import sys
sys.path.insert(0, '/opt/trn_rl_repo')

import numpy as np


def _ensure_ntff_hook_module():
    """Provide antenv.axon_hooks if the image lacks it (needed only when
    BASS_TRACE=1 requests NTFF profiling through run_bass_kernel_spmd)."""
    try:
        import antenv.axon_hooks  # noqa: F401
        return
    except Exception:
        pass
    import contextlib, ctypes, os, types

    mod = types.ModuleType("antenv.axon_hooks")
    state = {"hook": None, "tried": False}
    so_path = "/opt/axon/libaxon_pjrt.so"

    def _make_hook(path):
        lib = ctypes.CDLL(path)
        if not hasattr(lib, "axon_start_nrt_profile"):
            return None
        lib.axon_start_nrt_profile.argtypes = [
            ctypes.POINTER(ctypes.c_int64), ctypes.c_size_t]
        lib.axon_start_nrt_profile.restype = ctypes.c_int64
        lib.axon_stop_nrt_profile.argtypes = [ctypes.c_char_p]
        lib.axon_stop_nrt_profile.restype = ctypes.c_int64

        @contextlib.contextmanager
        def _hook(output_dir, device_ids):
            import jax
            jax.devices()
            if device_ids:
                ids = (ctypes.c_int64 * len(device_ids))(*device_ids)
                rc = lib.axon_start_nrt_profile(ids, len(device_ids))
            else:
                rc = lib.axon_start_nrt_profile(None, 0)
            if rc != 0:
                raise RuntimeError(f"axon_start_nrt_profile rc={rc}")
            try:
                yield
            finally:
                n = lib.axon_stop_nrt_profile(str(output_dir).encode())
                if n < 0:
                    raise RuntimeError(f"axon_stop_nrt_profile rc={n}")
                print(f"profile: {n} file(s) written to {output_dir}")

        return _hook

    def get_axon_ntff_profile_hook():
        if state["hook"] is None and not state["tried"]:
            state["tried"] = True
            if os.path.exists(so_path):
                try:
                    state["hook"] = _make_hook(so_path)
                except Exception:
                    state["hook"] = None
        return state["hook"]

    def set_axon_ntff_profile_hook(hook):
        state["hook"] = hook
        state["tried"] = True

    mod.get_axon_ntff_profile_hook = get_axon_ntff_profile_hook
    mod.set_axon_ntff_profile_hook = set_axon_ntff_profile_hook
    sys.modules["antenv.axon_hooks"] = mod


_ensure_ntff_hook_module()

NCORES = 8
B, N, D, E = 128, 2048, 64, 32768
BL = B // NCORES          # 16 subgraphs per core
NT = N // 128             # 16 node tiles
NCHUNK = 512              # bn-chunk: 4 node tiles for one subgraph
NG = N // NCHUNK          # 4 chunks per subgraph

_BUILD_CACHE = {}
LAST_RESULTS = None


def _build(zero_bias=False):
    key = ("nc", zero_bias)
    if key in _BUILD_CACHE:
        return _BUILD_CACHE[key]
    import concourse.bacc as bacc
    import concourse.tile as tile
    from concourse import mybir
    dt = mybir.dt
    f32, f32r = dt.float32, dt.float32r
    Relu = mybir.ActivationFunctionType.Relu
    Alu = mybir.AluOpType

    nc = bacc.Bacc("TRN2", target_bir_lowering=False, debug=False)

    Xc = nc.dram_tensor("Xc", [BL, N, D], f32, kind="ExternalInput").ap()
    St = nc.dram_tensor("St", [NT, 128, NT, 128], dt.uint8, kind="ExternalInput").ap()
    W1n = nc.dram_tensor("W1n", [D, D], f32, kind="ExternalInput").ap()
    B1n = nc.dram_tensor("B1n", [D, 1], f32, kind="ExternalInput").ap()
    W2n = nc.dram_tensor("W2n", [D, D], f32, kind="ExternalInput").ap()
    B2n = nc.dram_tensor("B2n", [D, 1], f32, kind="ExternalInput").ap()
    W1DD = nc.dram_tensor("W1DD", [128, 128], f32, kind="ExternalInput").ap()
    W2DD = nc.dram_tensor("W2DD", [128, 128], f32, kind="ExternalInput").ap()
    B1DD = nc.dram_tensor("B1DD", [128, 1], f32, kind="ExternalInput").ap()
    B2DD = nc.dram_tensor("B2DD", [128, 1], f32, kind="ExternalInput").ap()
    Ident = nc.dram_tensor("Ident", [128, 128], f32, kind="ExternalInput").ap()
    Out = nc.dram_tensor("Out", [BL, N, D], f32, kind="ExternalOutput").ap()

    with tile.TileContext(nc) as tc:
        with tc.tile_pool(name="const", bufs=1) as constp, \
             tc.tile_pool(name="resident", bufs=1) as resp, \
             tc.tile_pool(name="osb", bufs=3) as osbp, \
             tc.tile_pool(name="sslf", bufs=2) as sslfp, \
             tc.tile_pool(name="dram", bufs=1, space="DRAM") as dram:

            # ---- constants ----
            ident = constp.tile([128, 128], f32)
            nc.sync.dma_start(ident[:], Ident[:])
            ident_r = constp.tile([64, 64], f32r)
            nc.vector.tensor_copy(ident_r[:], ident[:64, :64])
            ident_r128 = constp.tile([128, 128], f32r)
            nc.vector.tensor_copy(ident_r128[:], ident[:])

            wdd_f32 = constp.tile([128, 2 * 128], f32)
            nc.sync.dma_start(wdd_f32[:, 0:128], W1DD[:])
            nc.sync.dma_start(wdd_f32[:, 128:256], W2DD[:])
            wdd_r = constp.tile([128, 2 * 128], f32r)
            nc.vector.tensor_copy(wdd_r[:], wdd_f32[:])
            w1dd, w2dd = wdd_r[:, 0:128], wdd_r[:, 128:256]
            bdd = constp.tile([128, 2], f32)
            nc.sync.dma_start(bdd[:, 0:1], B1DD[:])
            nc.sync.dma_start(bdd[:, 1:2], B2DD[:])
            b1dd, b2dd = bdd[:, 0:1], bdd[:, 1:2]

            w_f32 = constp.tile([D, 2 * D], f32)
            nc.sync.dma_start(w_f32[:, 0 * D:1 * D], W1n[:])
            nc.sync.dma_start(w_f32[:, 1 * D:2 * D], W2n[:])
            w_r = constp.tile([D, 2 * D], f32r)
            nc.vector.tensor_copy(w_r[:], w_f32[:])
            w1n, w2n = w_r[:, 0 * D:1 * D], w_r[:, 1 * D:2 * D]
            biases = constp.tile([D, 2], f32)
            nc.sync.dma_start(biases[:, 0:1], B1n[:])
            nc.sync.dma_start(biases[:, 1:2], B2n[:])
            b1n, b2n = biases[:, 0:1], biases[:, 1:2]

            # ---- resident tensors ----
            # h split by b-half for clean pass-1/pass-2 dependencies
            h_half = [
                resp.tile([128, NT, BL // 2, D], f32r, name=f"hh{i}")
                for i in range(2)
            ]

            xpn = resp.tile([128, NT, D], f32)     # partial max, natural layout
            xmn = resp.tile([128, NT, D], f32)     # global max, natural layout
            xmaxT = resp.tile([64, N], f32r)       # global max, transposed
            nodex = resp.tile([128, NT, D], f32)

            cin = dram.tile([128, NT * D], f32)
            crs = dram.tile([16, NT * D], f32)
            cout = dram.tile([128, NT * D], f32)

            def pass_c(psC, kts):
                """stage C: both b-halves for each kt"""
                for kt in kts:
                    sslu = sslfp.tile([128, NT, 128], dt.uint8, tag="sslu")
                    nc.sync.dma_start(sslu[:], St[kt])
                    ssl = sslfp.tile([128, NT, 128], f32r, tag="ssl")
                    nc.vector.tensor_copy(ssl[:], sslu[:])
                    for H in range(2):
                        pc = psC.tile([128, 512], f32, tag="pc")
                        for jt in range(NT):
                            nc.tensor.matmul(
                                pc[:],
                                ssl[:, jt, :],
                                h_half[H][:, jt, :, :].rearrange("p b d -> p (b d)"),
                                start=(jt == 0), stop=(jt == NT - 1))
                        osb = osbp.tile([128, 512], f32, tag="osb")
                        nc.scalar.activation(osb[:], pc[:], Relu)
                        nc.sync.dma_start(
                            Out[H * 8:(H + 1) * 8,
                                kt * 128:(kt + 1) * 128, :].rearrange(
                                "b p d -> p b d"),
                            osb[:].rearrange("p (b d) -> p b d", d=D))

            # ============ stage A: MLP_t in pair-chunks, by b-half ============
            with tc.tile_pool(name="xn", bufs=28) as xnp, \
                 tc.tile_pool(name="xt", bufs=4) as xtp, \
                 tc.tile_pool(name="mid", bufs=4) as midp, \
                 tc.tile_pool(name="htt", bufs=4) as http, \
                 tc.tile_pool(name="nbr", bufs=2) as nbrp, \
                 tc.tile_pool(name="psA", bufs=2, space="PSUM") as psA:

                def chunk(p, g):
                    b0 = 2 * p
                    H = p // 4
                    xn = xnp.tile([128, 4, 2, D], f32r, tag="xn")
                    for j in range(2):
                        nc.sync.dma_start(
                            xn[:, :, j, :],
                            Xc[b0 + j, g * NCHUNK:(g + 1) * NCHUNK, :].rearrange(
                                "(t p) d -> p t d", p=128).bitcast(f32r),
                        )
                    gsl = slice(4 * g, 4 * g + 4)
                    # partial max in natural layout (runs right behind the DMA)
                    if p == 0:
                        nc.vector.tensor_tensor(
                            xpn[:, gsl, :], xn[:, :, 0, :].bitcast(f32), xn[:, :, 1, :].bitcast(f32), Alu.max)
                    else:
                        nc.vector.tensor_tensor(
                            xpn[:, gsl, :], xpn[:, gsl, :], xn[:, :, 0, :].bitcast(f32), Alu.max)
                        nc.vector.tensor_tensor(
                            xpn[:, gsl, :], xpn[:, gsl, :], xn[:, :, 1, :].bitcast(f32), Alu.max)
                    # transpose -> [128 (b,d), 512 n]
                    tp = psA.tile([128, NCHUNK], f32r, tag="tp")
                    for t in range(4):
                        nc.tensor.transpose(
                            tp[:, t * 128:(t + 1) * 128],
                            xn[:, t, :, :].rearrange("p b d -> p (b d)"),
                            ident_r128[:])
                    xt = xtp.tile([128, NCHUNK], f32r, tag="xt")
                    nc.vector.tensor_copy(xt[:], tp[:])
                    # L1/L2 with block-diagonal weights (2 subgraphs at once)
                    l1p = psA.tile([128, NCHUNK], f32, tag="l1")
                    nc.tensor.matmul(l1p[:], w1dd, xt[:], start=True, stop=True)
                    mid = midp.tile([128, NCHUNK], f32r, tag="mid")
                    if zero_bias:
                        nc.vector.tensor_scalar_max(mid[:], l1p[:], 0.0)
                    else:
                        nc.scalar.activation(mid[:], l1p[:], Relu, bias=b1dd)
                    l2p = psA.tile([128, NCHUNK], f32, tag="l2")
                    nc.tensor.matmul(l2p[:], w2dd, mid[:], start=True, stop=True)
                    htt = http.tile([128, NCHUNK], f32r, tag="htt")
                    nc.scalar.activation(htt[:], l2p[:], Relu, bias=b2dd)
                    # transpose back: [128 n, (2b x 64d)] slabs into h_half
                    htp = psA.tile([128, 4, 2, D], f32r, tag="htp")
                    for t in range(4):
                        nc.tensor.transpose(
                            htp[:, t, :, :].rearrange("p b d -> p (b d)"),
                            htt[:, t * 128:(t + 1) * 128], ident_r128[:])
                    nc.scalar.activation(
                        h_half[H][:, gsl, (b0 % 8):(b0 % 8) + 2, :], htp[:], Relu)

                # ---- b-half 0 then b-half 1 chunk pipelines ----
                for g in range(NG):
                    for p in range(4):
                        chunk(p, g)
                for g in range(NG):
                    for p in range(4, 8):
                        chunk(p, g)

                # ---- pooled branch (high prio: jumps engine queues) ----
                nc.sync.dma_start(cin[:], xpn[:].rearrange("p a b -> p (a b)"))
                nc.gpsimd.collective_compute(
                    "ReduceScatter",
                    Alu.max,
                    replica_groups=[list(range(NCORES))],
                    ins=[cin[:].opt()],
                    outs=[crs[:].opt()],
                )
                nc.gpsimd.collective_compute(
                    "AllGather",
                    Alu.bypass,
                    replica_groups=[list(range(NCORES))],
                    ins=[crs[:].opt()],
                    outs=[cout[:].opt()],
                )
                nc.sync.dma_start(xmn[:].rearrange("p a b -> p (a b)"), cout[:])
                # transpose xmn -> xmaxT
                for q in range(NG):
                    tpn = psA.tile([64, NCHUNK], f32, tag="l1")
                    for t in range(4):
                        nc.tensor.transpose(
                            tpn[:, t * 128:(t + 1) * 128],
                            xmn[:, 4 * q + t, :], ident[:])
                    nc.vector.tensor_copy(
                        xmaxT[:, q * NCHUNK:(q + 1) * NCHUNK], tpn[:])
                # MLP_n
                for q in range(NG):
                    qs = slice(q * NCHUNK, (q + 1) * NCHUNK)
                    l1pn = psA.tile([64, NCHUNK], f32, tag="l1")
                    nc.tensor.matmul(l1pn[:], w1n, xmaxT[:, qs], start=True, stop=True)
                    midn = nbrp.tile([64, NCHUNK], f32r, tag="midn")
                    nc.scalar.activation(midn[:], l1pn[:], Relu, bias=b1n)
                    l2pn = psA.tile([64, NCHUNK], f32, tag="l2")
                    nc.tensor.matmul(l2pn[:], w2n, midn[:], start=True, stop=True)
                    httn = nbrp.tile([64, NCHUNK], f32r, tag="httn")
                    nc.scalar.activation(httn[:], l2pn[:], Relu, bias=b2n)
                    htpn = psA.tile([128, 4, D], f32r, tag="htp")
                    for t in range(4):
                        nc.tensor.transpose(
                            htpn[:, t, :],
                            httn[:, t * 128:(t + 1) * 128], ident_r[:])
                    nc.vector.tensor_copy(nodex[:, 4 * q:4 * q + 4, :], htpn[:])

                # h' = h + nodex for b-half 0
                for jt in range(NT):
                    nc.vector.tensor_tensor(
                        h_half[0][:, jt],
                        h_half[0][:, jt],
                        nodex[:, jt, None, :].broadcast_to((128, BL // 2, D)),
                        Alu.add)

                for jt in range(NT):
                    nc.vector.tensor_tensor(
                        h_half[1][:, jt],
                        h_half[1][:, jt],
                        nodex[:, jt, None, :].broadcast_to((128, BL // 2, D)),
                        Alu.add)

            # =================== stage C ===================
            with tc.tile_pool(name="psC", bufs=4, space="PSUM") as psC:
                pass_c(psC, range(NT))

    nc.compile()
    _BUILD_CACHE[key] = nc
    return nc


def kernel(X, edge_index, W1t, b1t, W2t, b2t, W1n, b1n, W2n, b2n):
    global LAST_RESULTS
    from concourse.bass_utils import run_bass_kernel_spmd

    zb = all(
        float(np.abs(np.asarray(v)).max()) == 0.0
        for v in (b1t, b2t, b1n, b2n))
    nc = _build(zero_bias=zb)

    X = np.ascontiguousarray(X, dtype=np.float32)
    # dense adjacency S[src, dst] = edge count, tiled [kt, jt, 128, 128]
    S = np.zeros((N, N), dtype=np.int32)
    np.add.at(S, (edge_index[0].astype(np.int64), edge_index[1].astype(np.int64)), 1)
    assert S.max() < 256
    St = np.ascontiguousarray(
        S.reshape(NT, 128, NT, 128).transpose(2, 1, 0, 3).astype(np.uint8))

    common = {
        "St": St,
        "W1n": np.ascontiguousarray(W1n, np.float32),
        "B1n": np.ascontiguousarray(b1n, np.float32).reshape(D, 1),
        "W2n": np.ascontiguousarray(W2n, np.float32),
        "B2n": np.ascontiguousarray(b2n, np.float32).reshape(D, 1),
        "W1DD": np.block([
            [np.asarray(W1t, np.float32), np.zeros((D, D), np.float32)],
            [np.zeros((D, D), np.float32), np.asarray(W1t, np.float32)]]),
        "W2DD": np.block([
            [np.asarray(W2t, np.float32), np.zeros((D, D), np.float32)],
            [np.zeros((D, D), np.float32), np.asarray(W2t, np.float32)]]),
        "B1DD": np.concatenate([np.asarray(b1t, np.float32).ravel()] * 2).reshape(128, 1),
        "B2DD": np.concatenate([np.asarray(b2t, np.float32).ravel()] * 2).reshape(128, 1),
        "Ident": np.eye(128, dtype=np.float32),
    }
    in_maps = [
        {"Xc": np.ascontiguousarray(X[c * BL:(c + 1) * BL]), **common}
        for c in range(NCORES)
    ]
    import os as _os
    _tc = list(range(NCORES)) if _os.environ.get("BASS_TRACE_ALL") else None
    res = run_bass_kernel_spmd(nc, in_maps, list(range(NCORES)), trace_cores=_tc)
    LAST_RESULTS = res
    out = np.empty((B, N, D), dtype=np.float32)
    for c in range(NCORES):
        out[c * BL:(c + 1) * BL] = res.results[c]["Out"]
    return out



# revision 9
# speedup vs baseline: 1.6088x; 1.6088x over previous
"""Trainium2 Bass kernel for DSS-GIN conv (gnn_message_passing).

Strategy (8 NeuronCores, B=128 subgraphs sharded 16/core):
  - Stream X in (f32), convert to bf16, running max over local subgraphs
    (DVE) and PE-transpose chunks to [bd, n] layout, all pipelined with DMA.
  - h = MLP_t(X) in bf16 via PE matmuls with 128x128 block-diagonal weights
    (2 subgraphs at a time), quantized to fp8 e4m3 and transposed back into
    a resident h8 [128, 16 jt, 1024] tile.
  - Pooled branch: partial max (bf16) -> ReduceScatter(max)+AllGather across
    the 8 cores, kicked as soon as X is fully loaded (~26us) so the wire
    time overlaps stage C; then nodex = MLP_n(xmax) -> fp8 nodex8.
  - Stage C: ret1 = S^T h as dense fp8 DoubleRow matmuls (256-contraction,
    2x PE rate; S counts are e4m3-exact).  Evictions staged in bf16;
    final DVE pass adds broadcast nodex2 = S^T nodex and streams f32 out.
"""
import sys
sys.path.insert(0, '/opt/trn_rl_repo')

import numpy as np


def _ensure_ntff_hook_module():
    """Provide antenv.axon_hooks if the image lacks it (needed only when
    BASS_TRACE=1 requests NTFF profiling through run_bass_kernel_spmd)."""
    try:
        import antenv.axon_hooks  # noqa: F401
        return
    except Exception:
        pass
    import contextlib, ctypes, os, types

    mod = types.ModuleType("antenv.axon_hooks")
    state = {"hook": None, "tried": False}
    so_path = "/opt/axon/libaxon_pjrt.so"

    def _make_hook(path):
        lib = ctypes.CDLL(path)
        if not hasattr(lib, "axon_start_nrt_profile"):
            return None
        lib.axon_start_nrt_profile.argtypes = [
            ctypes.POINTER(ctypes.c_int64), ctypes.c_size_t]
        lib.axon_start_nrt_profile.restype = ctypes.c_int64
        lib.axon_stop_nrt_profile.argtypes = [ctypes.c_char_p]
        lib.axon_stop_nrt_profile.restype = ctypes.c_int64

        @contextlib.contextmanager
        def _hook(output_dir, device_ids):
            import jax
            jax.devices()
            if device_ids:
                ids = (ctypes.c_int64 * len(device_ids))(*device_ids)
                rc = lib.axon_start_nrt_profile(ids, len(device_ids))
            else:
                rc = lib.axon_start_nrt_profile(None, 0)
            if rc != 0:
                raise RuntimeError(f"axon_start_nrt_profile rc={rc}")
            try:
                yield
            finally:
                n = lib.axon_stop_nrt_profile(str(output_dir).encode())
                if n < 0:
                    raise RuntimeError(f"axon_stop_nrt_profile rc={n}")
                print(f"profile: {n} file(s) written to {output_dir}")

        return _hook

    def get_axon_ntff_profile_hook():
        if state["hook"] is None and not state["tried"]:
            state["tried"] = True
            if os.path.exists(so_path):
                try:
                    state["hook"] = _make_hook(so_path)
                except Exception:
                    state["hook"] = None
        return state["hook"]

    def set_axon_ntff_profile_hook(hook):
        state["hook"] = hook
        state["tried"] = True

    mod.get_axon_ntff_profile_hook = get_axon_ntff_profile_hook
    mod.set_axon_ntff_profile_hook = set_axon_ntff_profile_hook
    sys.modules["antenv.axon_hooks"] = mod


_ensure_ntff_hook_module()

NCORES = 8
B, N, D, E = 128, 2048, 64, 32768
BL = B // NCORES          # 16 subgraphs per core
NT = N // 128             # 16 node tiles
NQ = NT // 2              # 8 src-tile pairs (DoubleRow contraction)
NG = 4                    # node tiles per 512-node chunk
K0 = 9                    # kt index at which the nodex2 chain starts

_BUILD_CACHE = {}
LAST_RESULTS = None


def _build():
    if "nc" in _BUILD_CACHE:
        return _BUILD_CACHE["nc"]
    import concourse.bacc as bacc
    import concourse.tile as tile
    from concourse import mybir
    dt = mybir.dt
    f32, bf16, f8 = dt.float32, dt.bfloat16, dt.float8e4
    Relu = mybir.ActivationFunctionType.Relu
    Alu = mybir.AluOpType
    DR = mybir.MatmulPerfMode.DoubleRow

    nc = bacc.Bacc("TRN2", target_bir_lowering=False, debug=False)

    Xc = nc.dram_tensor("Xc", [BL, N, D], f32, kind="ExternalInput").ap()
    St8 = nc.dram_tensor("St8", [NT, 128, NQ, 2, 128], dt.uint8,
                         kind="ExternalInput").ap()
    W1n = nc.dram_tensor("W1n", [D, D], f32, kind="ExternalInput").ap()
    B1n = nc.dram_tensor("B1n", [D, 1], f32, kind="ExternalInput").ap()
    W2n = nc.dram_tensor("W2n", [D, D], f32, kind="ExternalInput").ap()
    B2n = nc.dram_tensor("B2n", [D, 1], f32, kind="ExternalInput").ap()
    W1DD = nc.dram_tensor("W1DD", [128, 128], f32, kind="ExternalInput").ap()
    W2DD = nc.dram_tensor("W2DD", [128, 128], f32, kind="ExternalInput").ap()
    B1DD = nc.dram_tensor("B1DD", [128, 1], f32, kind="ExternalInput").ap()
    B2DD = nc.dram_tensor("B2DD", [128, 1], f32, kind="ExternalInput").ap()
    Ident = nc.dram_tensor("Ident", [128, 128], f32, kind="ExternalInput").ap()
    Out = nc.dram_tensor("Out", [BL, N, D], f32, kind="ExternalOutput").ap()

    with tile.TileContext(nc) as tc:
        with tc.tile_pool(name="const", bufs=1) as constp, \
             tc.tile_pool(name="resident", bufs=1) as resp, \
             tc.tile_pool(name="ssl", bufs=1) as sslp, \
             tc.tile_pool(name="stg", bufs=1) as stgp, \
             tc.tile_pool(name="osb", bufs=4) as osbp, \
             tc.tile_pool(name="dram", bufs=1, space="DRAM") as dram:

            # ---- constants ----
            identf = constp.tile([128, 128], f32)
            nc.sync.dma_start(identf[:], Ident[:])
            identb = constp.tile([128, 128], bf16)
            nc.vector.tensor_copy(identb[:], identf[:])

            wdd_f32 = constp.tile([128, 2, 128], f32)
            nc.sync.dma_start(wdd_f32[:, 0], W1DD[:])
            nc.sync.dma_start(wdd_f32[:, 1], W2DD[:])
            wddb = constp.tile([128, 2, 128], bf16)
            nc.vector.tensor_copy(wddb[:], wdd_f32[:])
            w1dd, w2dd = wddb[:, 0], wddb[:, 1]
            bdd = constp.tile([128, 2], f32)
            nc.sync.dma_start(bdd[:, 0:1], B1DD[:])
            nc.sync.dma_start(bdd[:, 1:2], B2DD[:])
            b1dd, b2dd = bdd[:, 0:1], bdd[:, 1:2]

            wn_f32 = constp.tile([D, 2, D], f32)
            nc.sync.dma_start(wn_f32[:, 0], W1n[:])
            nc.sync.dma_start(wn_f32[:, 1], W2n[:])
            wnb = constp.tile([D, 2, D], bf16)
            nc.vector.tensor_copy(wnb[:], wn_f32[:])
            w1n, w2n = wnb[:, 0], wnb[:, 1]
            bn = constp.tile([D, 2], f32)
            nc.sync.dma_start(bn[:, 0:1], B1n[:])
            nc.sync.dma_start(bn[:, 1:2], B2n[:])
            b1n, b2n = bn[:, 0:1], bn[:, 1:2]

            # ---- resident tensors ----
            h8 = resp.tile([128, NT, BL * D], f8)      # MLP_t out, fp8
            nodex8 = resp.tile([128, NT, D], f8)       # MLP_n out, fp8
            xpn = resp.tile([128, NT, D], bf16)        # local max over 16 subg
            xmn = resp.tile([128, NT, D], bf16)        # global max
            xmaxT = resp.tile([D, NT, 128], bf16)      # global max, transposed
            nodex2 = resp.tile([128, NT, D], f32)      # S^T nodex

            cin = dram.tile([128, NT * D], bf16)
            crs = dram.tile([128 // NCORES, NT * D], bf16)
            cout = dram.tile([128, NT * D], bf16)

            # S tiles (e4m3 bytes), all resident: stage C + nodex2 read them
            ssl = [sslp.tile([128, NQ, 2, 128], dt.uint8,
                             name=f"ssl{kt}", tag=f"ssl{kt}")
                   for kt in range(NT)]
            # staging for stage C evictions (finalized after nodex2)
            stg = [[stgp.tile([128, 512], bf16,
                              name=f"st{kt}_{hh}", tag=f"st{kt}_{hh}")
                    for hh in range(2)] for kt in range(NT)]

            # ============ stage A: stream X, max, MLP_t -> h8 ============
            with tc.tile_pool(name="xin", bufs=4) as xinp, \
                 tc.tile_pool(name="xb", bufs=4) as xbp, \
                 tc.tile_pool(name="xt", bufs=4) as xtp, \
                 tc.tile_pool(name="mid", bufs=3) as midp, \
                 tc.tile_pool(name="htt", bufs=3) as httpool, \
                 tc.tile_pool(name="psT", bufs=2, space="PSUM") as psT, \
                 tc.tile_pool(name="psA", bufs=2, space="PSUM") as psA:

                def chunk(p, g):
                    # subgraph pair p (2p, 2p+1), nodes g*512..(g+1)*512
                    xin = xinp.tile([128, NG, 2, D], f32, tag="xin")
                    for j in range(2):
                        nc.sync.dma_start(
                            xin[:, :, j, :],
                            Xc[2 * p + j,
                               g * 512:(g + 1) * 512, :].rearrange(
                                "(t p) d -> p t d", p=128))
                    xb = xbp.tile([128, NG, 2, D], bf16, tag="xb")
                    nc.vector.tensor_copy(xb[:], xin[:])
                    # running max over subgraphs (natural layout)
                    gsl = slice(4 * g, 4 * g + 4)
                    if p == 0:
                        nc.vector.tensor_tensor(
                            xpn[:, gsl, :], xb[:, :, 0, :], xb[:, :, 1, :],
                            Alu.max)
                    else:
                        nc.vector.tensor_tensor(
                            xpn[:, gsl, :], xpn[:, gsl, :], xb[:, :, 0, :],
                            Alu.max)
                        nc.vector.tensor_tensor(
                            xpn[:, gsl, :], xpn[:, gsl, :], xb[:, :, 1, :],
                            Alu.max)
                    # transpose -> [(2b,64d) part, 512 n]
                    tp = psT.tile([128, 512], bf16, tag="tp")
                    for t in range(4):
                        nc.tensor.transpose(
                            tp[:, t * 128:(t + 1) * 128],
                            xb[:, t, :, :].rearrange("p b d -> p (b d)"),
                            identb[:])
                    xt = xtp.tile([128, 512], bf16, tag="xt")
                    nc.vector.tensor_copy(xt[:], tp[:])
                    # L1/L2 with block-diagonal weights (2 subgraphs at once)
                    l1p = psA.tile([128, 512], f32, tag="l1")
                    nc.tensor.matmul(l1p[:], w1dd, xt[:], start=True, stop=True)
                    mid = midp.tile([128, 512], bf16, tag="mid")
                    nc.scalar.activation(mid[:], l1p[:], Relu, bias=b1dd)
                    l2p = psA.tile([128, 512], f32, tag="l2")
                    nc.tensor.matmul(l2p[:], w2dd, mid[:], start=True, stop=True)
                    httb = httpool.tile([128, 512], bf16, tag="httb")
                    nc.scalar.activation(httb[:], l2p[:], Relu, bias=b2dd)
                    # transpose back (bf16): [n part, (2b x 64d)], then the
                    # PSUM->SBUF copy quantizes to fp8
                    htp = psT.tile([128, NG, 128], bf16, tag="htp")
                    for t in range(4):
                        nc.tensor.transpose(
                            htp[:, t, :], httb[:, t * 128:(t + 1) * 128],
                            identb[:])
                    nc.vector.tensor_copy(
                        h8[:, gsl, 128 * p:128 * (p + 1)], htp[:])

                for p in range(8):
                    for g in range(NG):
                        chunk(p, g)

                # ---- kick the collective as soon as the local max is done
                nc.sync.dma_start(cin[:], xpn[:].rearrange("p a b -> p (a b)"))
                nc.gpsimd.collective_compute(
                    "ReduceScatter", Alu.max,
                    replica_groups=[list(range(NCORES))],
                    ins=[cin[:].opt()], outs=[crs[:].opt()])
                nc.gpsimd.collective_compute(
                    "AllGather", Alu.bypass,
                    replica_groups=[list(range(NCORES))],
                    ins=[crs[:].opt()], outs=[cout[:].opt()])
                nc.sync.dma_start(
                    xmn[:].rearrange("p a b -> p (a b)"), cout[:])

                # S tile loads (emitted after X DMAs so X wins the queues)
                for kt in range(NT):
                    nc.sync.dma_start(ssl[kt][:], St8[kt])

            # =========== stage C + nodex2 chain ===========
            with tc.tile_pool(name="psC", bufs=3, space="PSUM") as psC, \
                 tc.tile_pool(name="psN", bufs=1, space="PSUM") as psN, \
                 tc.tile_pool(name="psPN", bufs=1, space="PSUM") as psPN, \
                 tc.tile_pool(name="nmid", bufs=2) as nmidp:

                def nodex2_prep():
                    # xmn -> xmaxT (bf16 transposes)
                    for q in range(NG):
                        tpn = psN.tile([D, 512], bf16, tag="tpn")
                        for t in range(4):
                            nc.tensor.transpose(
                                tpn[:, t * 128:(t + 1) * 128],
                                xmn[:, 4 * q + t, :], identb[:])
                        nc.vector.tensor_copy(
                            xmaxT[:, 4 * q:4 * q + 4, :].rearrange(
                                "p a b -> p (a b)"), tpn[:])
                    # MLP_n: nodexT = relu(W2n^T relu(W1n^T xmaxT + b1) + b2)
                    for q in range(NG):
                        qs = xmaxT[:, 4 * q:4 * q + 4, :].rearrange(
                            "p a b -> p (a b)")
                        l1pn = psN.tile([D, 512], f32, tag="l1n")
                        nc.tensor.matmul(l1pn[:], w1n, qs, start=True, stop=True)
                        midn = nmidp.tile([D, 512], bf16, tag="midn")
                        nc.scalar.activation(midn[:], l1pn[:], Relu, bias=b1n)
                        l2pn = psN.tile([D, 512], f32, tag="l2n")
                        nc.tensor.matmul(l2pn[:], w2n, midn[:],
                                         start=True, stop=True)
                        nxtb = nmidp.tile([D, 512], bf16, tag="nxtb")
                        nc.scalar.activation(nxtb[:], l2pn[:], Relu, bias=b2n)
                        # transpose back into nodex8 (quantize on copy)
                        ntp = psN.tile([128, NG, D], bf16, tag="ntp")
                        for t in range(4):
                            nc.tensor.transpose(
                                ntp[:, t, :], nxtb[:, t * 128:(t + 1) * 128],
                                identb[0:D, 0:D])
                        nc.vector.tensor_copy(
                            nodex8[:, 4 * q:4 * q + 4, :], ntp[:])

                def nodex2_mm(kt):
                    # nodex2[kt] = S^T nodex (DoubleRow over src-tile pairs)
                    s8 = ssl[kt][:].bitcast(f8)
                    pn = psPN.tile([128, D], f32, tag="pn")
                    for q in range(NQ):
                        nc.tensor.matmul(
                            pn[:], s8[:, q], nodex8[:, 2 * q:2 * q + 2, :],
                            start=(q == 0), stop=(q == NQ - 1), perf_mode=DR)
                    nc.scalar.activation(nodex2[:, kt, :], pn[:], Relu)

                ndone = 0
                for kt in range(NT):
                    if kt == K0:
                        nodex2_prep()
                    s8 = ssl[kt][:].bitcast(f8)
                    for hh in range(2):
                        pc = psC.tile([128, 512], f32, tag="pc")
                        for q in range(NQ):
                            nc.tensor.matmul(
                                pc[:], s8[:, q],
                                h8[:, 2 * q:2 * q + 2,
                                   hh * 512:(hh + 1) * 512],
                                start=(q == 0), stop=(q == NQ - 1),
                                perf_mode=DR)
                        nc.scalar.activation(stg[kt][hh][:], pc[:], Relu)
                    if kt > K0:
                        for _ in range(3):
                            if ndone < NT:
                                nodex2_mm(ndone)
                                ndone += 1
                while ndone < NT:
                    nodex2_mm(ndone)
                    ndone += 1

                # ---- finalize: add broadcast nodex2, stream out ----
                for kt in range(NT):
                    for hh in range(2):
                        osb = osbp.tile([128, 8, D], f32, tag="osb")
                        nc.vector.tensor_tensor(
                            osb[:],
                            stg[kt][hh][:].rearrange(
                                "p (b d) -> p b d", d=D),
                            nodex2[:, kt, None, :].broadcast_to((128, 8, D)),
                            Alu.add)
                        nc.sync.dma_start(
                            Out[hh * 8:(hh + 1) * 8,
                                kt * 128:(kt + 1) * 128, :].rearrange(
                                "b p d -> p b d"),
                            osb[:])

    nc.compile()
    _BUILD_CACHE["nc"] = nc
    return nc


def make_in_maps(X, edge_index, W1t, b1t, W2t, b2t, W1n, b1n, W2n, b2n):
    import ml_dtypes
    X = np.ascontiguousarray(X, dtype=np.float32)
    # dense adjacency S[src, dst] = edge count, e4m3-encoded (counts are
    # small ints, exactly representable), tiled for DoubleRow:
    # St8[kt, p, q, i, f] = S[q*256 + i*128 + p, kt*128 + f]
    S = np.zeros((N, N), dtype=np.int32)
    np.add.at(S, (edge_index[0].astype(np.int64),
                  edge_index[1].astype(np.int64)), 1)
    assert S.max() <= 16
    S8 = S.astype(ml_dtypes.float8_e4m3).view(np.uint8)
    St8 = np.ascontiguousarray(
        S8.reshape(NQ, 2, 128, NT, 128).transpose(3, 2, 0, 1, 4))

    common = {
        "St8": St8,
        "W1n": np.ascontiguousarray(W1n, np.float32),
        "B1n": np.ascontiguousarray(b1n, np.float32).reshape(D, 1),
        "W2n": np.ascontiguousarray(W2n, np.float32),
        "B2n": np.ascontiguousarray(b2n, np.float32).reshape(D, 1),
        "W1DD": np.block([
            [np.asarray(W1t, np.float32), np.zeros((D, D), np.float32)],
            [np.zeros((D, D), np.float32), np.asarray(W1t, np.float32)]]),
        "W2DD": np.block([
            [np.asarray(W2t, np.float32), np.zeros((D, D), np.float32)],
            [np.zeros((D, D), np.float32), np.asarray(W2t, np.float32)]]),
        "B1DD": np.concatenate(
            [np.asarray(b1t, np.float32).ravel()] * 2).reshape(128, 1),
        "B2DD": np.concatenate(
            [np.asarray(b2t, np.float32).ravel()] * 2).reshape(128, 1),
        "Ident": np.eye(128, dtype=np.float32),
    }
    return [
        {"Xc": np.ascontiguousarray(X[c * BL:(c + 1) * BL]), **common}
        for c in range(NCORES)
    ]


def kernel(X, edge_index, W1t, b1t, W2t, b2t, W1n, b1n, W2n, b2n):
    global LAST_RESULTS
    from concourse.bass_utils import run_bass_kernel_spmd

    nc = _build()
    in_maps = make_in_maps(X, edge_index, W1t, b1t, W2t, b2t,
                           W1n, b1n, W2n, b2n)
    import os as _os
    _tc = list(range(NCORES)) if _os.environ.get("BASS_TRACE_ALL") else None
    res = run_bass_kernel_spmd(nc, in_maps, list(range(NCORES)),
                               trace_cores=_tc)
    LAST_RESULTS = res
    out = np.empty((B, N, D), dtype=np.float32)
    for c in range(NCORES):
        out[c * BL:(c + 1) * BL] = res.results[c]["Out"]
    return out
